# revision 7
# baseline (speedup 1.0000x reference)
"""Trainium2 Bass kernel for nn_ClassLoss_11828339933550.

YOLO-style classification loss over 3 scales:
  loss = sum_s sum_b CE_mean(log_softmax(out_s[b,...,5:]), gt_scatter(targets[b])) / B

Strategy (data-parallel over batch, 2 batches per core on 8 cores):
  Host: build per-scale ground-truth class maps from `targets` (tiny [16,100,5]
  tensor, last-wins scatter), derive a per-row weight vector w (1/denom at
  masked rows, else 0) plus a compact list of (masked row, class) pairs.
  Device (per core, streaming all of its 41 MB shard):
    - stream pred rows [128, K*80] tiles; ACT exp in-place; DVE grouped
      reduce -> per-row sumexp
    - ACT ln over the per-row sumexp buffer; DVE tensor_tensor_reduce with the
      streamed w vector -> S1 = sum_r w_r * lse_r
    - compact gathered masked rows: one-hot select via iota==cls, weighted;
      TTR -> S2 = sum_r w_r * pred[r, cls_r]
  Host: loss = sum_cores(S1 - S2) / B.
"""

import numpy as np

import concourse.bass as bass
import concourse.tile as tile
from concourse import mybir
from concourse.bass_utils import run_bass_kernel_spmd

# Problem constants (hardcoded per spec nn_ClassLoss_11828339933550)
B, T, A, C = 16, 100, 3, 80
GRIDS = (128, 64, 32)
IGNORE = -100
NCORES = 8
BPC = B // NCORES  # batches per core = 2

ROWS_PER_BATCH = A * sum(g * g for g in GRIDS)  # 64512
ROWS_PER_CORE = BPC * ROWS_PER_BATCH  # 129024
P = 128
K = 48  # rows per partition per tile
F = K * C  # 3840 floats per partition per tile
NT = ROWS_PER_CORE // (P * K)  # 21 tiles
NTW = NT * K  # 1008 per-row columns per partition
NG = 16  # compact gather tiles of [128, C] -> capacity 2048 masked rows

_DT = mybir.dt.float32

LAST_RESULTS = None  # debugging: last BassKernelResults (used by test.py)

# The walrus build in this container encodes at most _MAXW sync-wait commands
# per instruction ("Too many sync wait commands" in codegen otherwise). The
# Tile scheduler merges waits onto single instructions (e.g. the kernel-tail
# drain waits on every DMA semaphore at once), so split any excess waits onto
# preceding wait-only NoOps on the same engine — the sequencer executes them
# in order, which is semantically identical.
_MAXW = 1


def _split_excess_waits(bir: bytes) -> bytes:
    import json as _json

    m = _json.loads(bir)
    n = 0
    for fn in m["functions"]:
        for bb in fn["blocks"]:
            new_instrs = []
            for ins in bb.get("instructions", []):
                si = ins.get("sync_info")
                waits = (si or {}).get("on_wait") or []
                if si is not None and len(waits) > _MAXW:
                    excess = waits[:-_MAXW]
                    si["on_wait"] = waits[-_MAXW:]
                    for i in range(0, len(excess), _MAXW):
                        n += 1
                        new_instrs.append(
                            {
                                "engine": ins["engine"],
                                "ins": [],
                                "outs": [],
                                "name": f"waitsplit-{n}",
                                "opcode": "NoOp",
                                "sync_info": {
                                    "on_update": [],
                                    "on_wait": excess[i : i + _MAXW],
                                },
                            }
                        )
                new_instrs.append(ins)
            bb["instructions"] = new_instrs
    return _json.dumps(m).encode()


class _Bass(bass.Bass):
    def to_json_bytes(self):
        return _split_excess_waits(super().to_json_bytes())


def _build_gt_flat(targets_b, H, W):
    """Per-batch gt map -> flattened (H, W, A) class vector, IGNORE elsewhere."""
    valid = ~np.all(targets_b == 0.0, axis=1)
    rows = (targets_b[:, 2] * H).astype(np.int32)
    cols = (targets_b[:, 1] * W).astype(np.int32)
    cls = targets_b[:, 0].astype(np.int32)
    gt = np.full((H, W), IGNORE, dtype=np.int32)
    idx = np.where(valid)[0]
    gt[rows[idx], cols[idx]] = cls[idx]  # sequential last-wins, like index_put_
    return np.broadcast_to(gt[:, :, None], (H, W, A)).reshape(-1)


def _build_kernel():
    nc = _Bass("TRN2", target_bir_lowering=False, debug=False)

    xs = nc.declare_dram_parameter("xs", [NT * P, F], _DT, isOutput=False)
    wt = nc.declare_dram_parameter("wt", [P, NTW], _DT, isOutput=False)
    gp = nc.declare_dram_parameter("gp", [NG * P, C], _DT, isOutput=False)
    gc = nc.declare_dram_parameter("gc", [P, NG], _DT, isOutput=False)
    gw = nc.declare_dram_parameter("gw", [P, NG], _DT, isOutput=False)
    res = nc.declare_dram_parameter("res", [P, 2], _DT, isOutput=True)

    with tile.TileContext(nc) as tc:
        with (
            tc.tile_pool(name="singles", bufs=1) as singles,
            tc.tile_pool(name="xpool", bufs=4) as xpool,
            tc.tile_pool(name="gpool", bufs=2) as gpool,
        ):
            sebuf = singles.tile([P, NTW], _DT)
            wt_sb = singles.tile([P, NTW], _DT)
            gc_sb = singles.tile([P, NG], _DT)
            gw_sb = singles.tile([P, NG], _DT)
            s2buf = singles.tile([P, NG], _DT)
            iota80 = singles.tile([P, C], _DT)
            restile = singles.tile([P, 2], _DT)
            scrbig = singles.tile([P, NTW], _DT)

            nc.sync.dma_start(out=wt_sb[:], in_=wt[:, :])
            nc.sync.dma_start(out=gc_sb[:], in_=gc[:, :])
            nc.sync.dma_start(out=gw_sb[:], in_=gw[:, :])
            nc.gpsimd.iota(
                iota80[:],
                pattern=[[1, C]],
                base=0,
                channel_multiplier=0,
                allow_small_or_imprecise_dtypes=True,
            )

            # Main stream: per-row sumexp of exp(logits)
            for t in range(NT):
                xtile = xpool.tile([P, F], _DT)
                nc.sync.dma_start(out=xtile[:], in_=xs[t * P : (t + 1) * P, :])
                nc.scalar.activation(
                    out=xtile[:], in_=xtile[:], func=mybir.ActivationFunctionType.Exp
                )
                nc.vector.tensor_reduce(
                    out=sebuf[:, t * K : (t + 1) * K],
                    in_=xtile[:].rearrange("p (k c) -> p k c", k=K),
                    axis=mybir.AxisListType.X,
                    op=mybir.AluOpType.add,
                )

            # Compact select: S2 contributions per gathered masked row
            for j in range(NG):
                gtile = gpool.tile([P, C], _DT, tag="gtile")
                nc.sync.dma_start(out=gtile[:], in_=gp[j * P : (j + 1) * P, :])
                woh = gpool.tile([P, C], _DT, tag="woh")
                nc.vector.tensor_scalar(
                    out=woh[:],
                    in0=iota80[:],
                    scalar1=gc_sb[:, j : j + 1],
                    scalar2=gw_sb[:, j : j + 1],
                    op0=mybir.AluOpType.is_equal,
                    op1=mybir.AluOpType.mult,
                )
                scr = gpool.tile([P, C], _DT, tag="scr")
                nc.vector.tensor_tensor(
                    out=scr[:],
                    in0=woh[:],
                    in1=gtile[:],
                    op=mybir.AluOpType.mult,
                )
                nc.vector.tensor_reduce(
                    out=s2buf[:, j : j + 1],
                    in_=scr[:],
                    axis=mybir.AxisListType.X,
                    op=mybir.AluOpType.add,
                )

            # Final: S1 = sum w * ln(sumexp); S2 = sum of compact accums
            nc.scalar.activation(
                out=sebuf[:], in_=sebuf[:], func=mybir.ActivationFunctionType.Ln
            )
            nc.vector.tensor_tensor(
                out=scrbig[:],
                in0=sebuf[:],
                in1=wt_sb[:],
                op=mybir.AluOpType.mult,
            )
            nc.vector.tensor_reduce(
                out=restile[:, 0:1],
                in_=scrbig[:],
                axis=mybir.AxisListType.X,
                op=mybir.AluOpType.add,
            )
            nc.vector.tensor_reduce(
                out=restile[:, 1:2],
                in_=s2buf[:],
                axis=mybir.AxisListType.X,
                op=mybir.AluOpType.add,
            )
            nc.sync.dma_start(out=res[:, :], in_=restile[:])

    return nc


def _prep_core_inputs(core, outs, targets):
    """Build the per-core input map (pred shard + weights + compact gather)."""
    pred_segs = []
    w_segs = []
    cls_segs = []
    for b in range(BPC * core, BPC * core + BPC):
        for si, H in enumerate(GRIDS):
            o = outs[si][b]  # [A, H, W, 85]
            pred_segs.append(np.ascontiguousarray(o[..., 5:]).reshape(-1, C))
            gt_flat = _build_gt_flat(targets[b], H, H)
            mask = gt_flat != IGNORE
            denom = max(int(mask.sum()), 1)
            w_segs.append(mask.astype(np.float32) / np.float32(denom))
            cls_segs.append(gt_flat)

    pred = np.concatenate(pred_segs, axis=0)  # [ROWS_PER_CORE, C] f32
    w_flat = np.concatenate(w_segs)  # [ROWS_PER_CORE]
    cls_flat = np.concatenate(cls_segs)  # [ROWS_PER_CORE] int32 (IGNORE at unmasked)

    xs = np.ascontiguousarray(pred.reshape(NT * P, F))
    wt = np.ascontiguousarray(
        w_flat.reshape(NT, P, K).transpose(1, 0, 2).reshape(P, NTW)
    )

    midx = np.where(w_flat > 0)[0]
    nm = len(midx)
    assert nm <= NG * P, f"masked rows {nm} exceed compact capacity"
    gp = np.zeros((NG * P, C), dtype=np.float32)
    gp[:nm] = pred[midx]
    gcw = np.zeros(NG * P, dtype=np.float32)
    gcl = np.zeros(NG * P, dtype=np.float32)
    gcw[:nm] = w_flat[midx]
    gcl[:nm] = cls_flat[midx].astype(np.float32)
    gc = np.ascontiguousarray(gcl.reshape(NG, P).T)
    gw = np.ascontiguousarray(gcw.reshape(NG, P).T)

    return {"xs": xs, "wt": wt, "gp": gp, "gc": gc, "gw": gw}


def kernel(out0, out1, out2, targets):
    out0 = np.asarray(out0, dtype=np.float32)
    out1 = np.asarray(out1, dtype=np.float32)
    out2 = np.asarray(out2, dtype=np.float32)
    targets = np.asarray(targets, dtype=np.float32)
    outs = (out0, out1, out2)

    in_maps = [_prep_core_inputs(c, outs, targets) for c in range(NCORES)]

    nc = _build_kernel()
    br = run_bass_kernel_spmd(nc, in_maps, list(range(NCORES)))
    global LAST_RESULTS
    LAST_RESULTS = br
    results = br.results

    total = 0.0
    for c in range(NCORES):
        r = np.asarray(results[c]["res"], dtype=np.float64)
        total += r[:, 0].sum() - r[:, 1].sum()
    return np.asarray(total / B, dtype=np.float32)


# revision 15
# speedup vs baseline: 1.0720x; 1.0720x over previous
"""Trainium2 Bass kernel for nn_ClassLoss_11828339933550.

YOLO-style classification loss over 3 scales:
  loss = sum_s sum_b CE_mean(log_softmax(out_s[b,...,5:]), gt_scatter(targets[b])) / B

Strategy (data-parallel over batch, 2 batches per core on 8 cores):
  Host: build per-scale ground-truth class maps from `targets` (tiny [16,100,5]
  tensor, last-wins scatter), derive a per-row weight vector w (1/denom at
  masked rows, else 0) plus a compact list of (masked row, class) pairs.
  Device (per core, streaming all of its 41 MB shard):
    - stream pred rows [128, K*80] tiles; ACT exp in-place; DVE grouped
      reduce -> per-row sumexp
    - ACT ln over the per-row sumexp buffer; DVE tensor_tensor_reduce with the
      streamed w vector -> S1 = sum_r w_r * lse_r
    - compact gathered masked rows: one-hot select via iota==cls, weighted;
      TTR -> S2 = sum_r w_r * pred[r, cls_r]
  Host: loss = sum_cores(S1 - S2) / B.
"""

import ml_dtypes
import numpy as np

import concourse.bass as bass
import concourse.tile as tile
from concourse import mybir
from concourse.bass_utils import run_bass_kernel_spmd

# Problem constants (hardcoded per spec nn_ClassLoss_11828339933550)
B, T, A, C = 16, 100, 3, 80
GRIDS = (128, 64, 32)
IGNORE = -100
NCORES = 8
BPC = B // NCORES  # batches per core = 2

ROWS_PER_BATCH = A * sum(g * g for g in GRIDS)  # 64512
ROWS_PER_CORE = BPC * ROWS_PER_BATCH  # 129024
P = 128
K = 48  # rows per partition per tile
F = K * C  # 3840 floats per partition per tile
NT = ROWS_PER_CORE // (P * K)  # 21 tiles
NTW = NT * K  # 1008 per-row columns per partition
NG = 16  # compact gather tiles of [128, C] -> capacity 2048 masked rows

_DT = mybir.dt.float32
# Streamed logits travel as bf16: halves HBM traffic, and 16-bit dtypes let
# the DVE grouped reduce run in 2x mode. The lse accumulation stays fp32
# internally (DVE/ACT compute fp32); the tiny gathered class-logit path stays
# full fp32. Measured end-to-end rel err stays ~1e-5.
_DT_X = mybir.dt.bfloat16

LAST_RESULTS = None  # debugging: last BassKernelResults (used by test.py)

# The walrus build in this container encodes at most _MAXW sync-wait commands
# per instruction ("Too many sync wait commands" in codegen otherwise). The
# Tile scheduler merges waits onto single instructions (e.g. the kernel-tail
# drain waits on every DMA semaphore at once), so split any excess waits onto
# preceding wait-only NoOps on the same engine — the sequencer executes them
# in order, which is semantically identical.
_MAXW = 1


def _split_excess_waits(bir: bytes) -> bytes:
    import json as _json

    m = _json.loads(bir)
    n = 0
    for fn in m["functions"]:
        for bb in fn["blocks"]:
            new_instrs = []
            for ins in bb.get("instructions", []):
                si = ins.get("sync_info")
                waits = (si or {}).get("on_wait") or []
                if si is not None and len(waits) > _MAXW:
                    excess = waits[:-_MAXW]
                    si["on_wait"] = waits[-_MAXW:]
                    for i in range(0, len(excess), _MAXW):
                        n += 1
                        new_instrs.append(
                            {
                                "engine": ins["engine"],
                                "ins": [],
                                "outs": [],
                                "name": f"waitsplit-{n}",
                                "opcode": "NoOp",
                                "sync_info": {
                                    "on_update": [],
                                    "on_wait": excess[i : i + _MAXW],
                                },
                            }
                        )
                new_instrs.append(ins)
            bb["instructions"] = new_instrs
    return _json.dumps(m).encode()


class _Bass(bass.Bass):
    def to_json_bytes(self):
        return _split_excess_waits(super().to_json_bytes())


def _build_gt_flat(targets_b, H, W):
    """Per-batch gt map -> flattened (H, W, A) class vector, IGNORE elsewhere."""
    valid = ~np.all(targets_b == 0.0, axis=1)
    rows = (targets_b[:, 2] * H).astype(np.int32)
    cols = (targets_b[:, 1] * W).astype(np.int32)
    cls = targets_b[:, 0].astype(np.int32)
    gt = np.full((H, W), IGNORE, dtype=np.int32)
    idx = np.where(valid)[0]
    gt[rows[idx], cols[idx]] = cls[idx]  # sequential last-wins, like index_put_
    return np.broadcast_to(gt[:, :, None], (H, W, A)).reshape(-1)


def _build_kernel():
    nc = _Bass("TRN2", target_bir_lowering=False, debug=False)

    xs = nc.declare_dram_parameter("xs", [NT * P, F], _DT_X, isOutput=False)
    wt = nc.declare_dram_parameter("wt", [P, NTW], _DT, isOutput=False)
    gp = nc.declare_dram_parameter("gp", [NG * P, C], _DT, isOutput=False)
    gc = nc.declare_dram_parameter("gc", [P, NG], _DT, isOutput=False)
    gw = nc.declare_dram_parameter("gw", [P, NG], _DT, isOutput=False)
    res = nc.declare_dram_parameter("res", [P, 2], _DT, isOutput=True)

    with tile.TileContext(nc) as tc:
        with (
            tc.tile_pool(name="singles", bufs=1) as singles,
            tc.tile_pool(name="xpool", bufs=4) as xpool,
            tc.tile_pool(name="gpool", bufs=2) as gpool,
        ):
            sebuf = singles.tile([P, NTW], _DT_X)
            logse = singles.tile([P, NTW], _DT)
            wt_sb = singles.tile([P, NTW], _DT)
            gc_sb = singles.tile([P, NG], _DT)
            gw_sb = singles.tile([P, NG], _DT)
            s2buf = singles.tile([P, NG], _DT)
            iota80 = singles.tile([P, C], _DT)
            restile = singles.tile([P, 2], _DT)
            scrbig = singles.tile([P, NTW], _DT)

            nc.sync.dma_start(out=wt_sb[:], in_=wt[:, :])
            nc.sync.dma_start(out=gc_sb[:], in_=gc[:, :])
            nc.sync.dma_start(out=gw_sb[:], in_=gw[:, :])
            nc.gpsimd.iota(
                iota80[:],
                pattern=[[1, C]],
                base=0,
                channel_multiplier=0,
                allow_small_or_imprecise_dtypes=True,
            )

            # Main stream: per-row sumexp of exp(logits)
            for t in range(NT):
                xtile = xpool.tile([P, F], _DT_X)
                nc.sync.dma_start(out=xtile[:], in_=xs[t * P : (t + 1) * P, :])
                nc.scalar.activation(
                    out=xtile[:], in_=xtile[:], func=mybir.ActivationFunctionType.Exp
                )
                # bf16 sumexp output: DVE accumulates fp32 internally; the
                # bf16 store costs ~0.2% relative on sumexp (~2e-6 on the
                # final loss) and enables the 2x DVE perf mode.
                with nc.allow_low_precision(reason="bf16 sumexp store, fp32 accum"):
                    nc.vector.tensor_reduce(
                        out=sebuf[:, t * K : (t + 1) * K],
                        in_=xtile[:].rearrange("p (k c) -> p k c", k=K),
                        axis=mybir.AxisListType.X,
                        op=mybir.AluOpType.add,
                    )

            # Compact select: S2 contributions per gathered masked row
            for j in range(NG):
                gtile = gpool.tile([P, C], _DT, tag="gtile")
                nc.sync.dma_start(out=gtile[:], in_=gp[j * P : (j + 1) * P, :])
                woh = gpool.tile([P, C], _DT, tag="woh")
                nc.vector.tensor_scalar(
                    out=woh[:],
                    in0=iota80[:],
                    scalar1=gc_sb[:, j : j + 1],
                    scalar2=gw_sb[:, j : j + 1],
                    op0=mybir.AluOpType.is_equal,
                    op1=mybir.AluOpType.mult,
                )
                scr = gpool.tile([P, C], _DT, tag="scr")
                nc.vector.tensor_tensor(
                    out=scr[:],
                    in0=woh[:],
                    in1=gtile[:],
                    op=mybir.AluOpType.mult,
                )
                nc.vector.tensor_reduce(
                    out=s2buf[:, j : j + 1],
                    in_=scr[:],
                    axis=mybir.AxisListType.X,
                    op=mybir.AluOpType.add,
                )

            # Final: S1 = sum w * ln(sumexp); S2 = sum of compact accums
            nc.scalar.activation(
                out=logse[:], in_=sebuf[:], func=mybir.ActivationFunctionType.Ln
            )
            nc.vector.tensor_tensor(
                out=scrbig[:],
                in0=logse[:],
                in1=wt_sb[:],
                op=mybir.AluOpType.mult,
            )
            nc.vector.tensor_reduce(
                out=restile[:, 0:1],
                in_=scrbig[:],
                axis=mybir.AxisListType.X,
                op=mybir.AluOpType.add,
            )
            nc.vector.tensor_reduce(
                out=restile[:, 1:2],
                in_=s2buf[:],
                axis=mybir.AxisListType.X,
                op=mybir.AluOpType.add,
            )
            nc.sync.dma_start(out=res[:, :], in_=restile[:])

    return nc


def _prep_core_inputs(core, outs, targets):
    """Build the per-core input map (pred shard + weights + compact gather)."""
    pred_segs = []
    w_segs = []
    cls_segs = []
    for b in range(BPC * core, BPC * core + BPC):
        for si, H in enumerate(GRIDS):
            o = outs[si][b]  # [A, H, W, 85]
            pred_segs.append(np.ascontiguousarray(o[..., 5:]).reshape(-1, C))
            gt_flat = _build_gt_flat(targets[b], H, H)
            mask = gt_flat != IGNORE
            denom = max(int(mask.sum()), 1)
            w_segs.append(mask.astype(np.float32) / np.float32(denom))
            cls_segs.append(gt_flat)

    pred = np.concatenate(pred_segs, axis=0)  # [ROWS_PER_CORE, C] f32
    w_flat = np.concatenate(w_segs)  # [ROWS_PER_CORE]
    cls_flat = np.concatenate(cls_segs)  # [ROWS_PER_CORE] int32 (IGNORE at unmasked)

    xs = np.ascontiguousarray(pred.reshape(NT * P, F)).astype(ml_dtypes.bfloat16)
    wt = np.ascontiguousarray(
        w_flat.reshape(NT, P, K).transpose(1, 0, 2).reshape(P, NTW)
    )

    midx = np.where(w_flat > 0)[0]
    nm = len(midx)
    assert nm <= NG * P, f"masked rows {nm} exceed compact capacity"
    gp = np.zeros((NG * P, C), dtype=np.float32)
    gp[:nm] = pred[midx]
    gcw = np.zeros(NG * P, dtype=np.float32)
    gcl = np.zeros(NG * P, dtype=np.float32)
    gcw[:nm] = w_flat[midx]
    gcl[:nm] = cls_flat[midx].astype(np.float32)
    gc = np.ascontiguousarray(gcl.reshape(NG, P).T)
    gw = np.ascontiguousarray(gcw.reshape(NG, P).T)

    return {"xs": xs, "wt": wt, "gp": gp, "gc": gc, "gw": gw}


def kernel(out0, out1, out2, targets):
    out0 = np.asarray(out0, dtype=np.float32)
    out1 = np.asarray(out1, dtype=np.float32)
    out2 = np.asarray(out2, dtype=np.float32)
    targets = np.asarray(targets, dtype=np.float32)
    outs = (out0, out1, out2)

    in_maps = [_prep_core_inputs(c, outs, targets) for c in range(NCORES)]

    nc = _build_kernel()
    br = run_bass_kernel_spmd(nc, in_maps, list(range(NCORES)))
    global LAST_RESULTS
    LAST_RESULTS = br
    results = br.results

    total = 0.0
    for c in range(NCORES):
        r = np.asarray(results[c]["res"], dtype=np.float64)
        total += r[:, 0].sum() - r[:, 1].sum()
    return np.asarray(total / B, dtype=np.float32)


# revision 17
# speedup vs baseline: 1.1927x; 1.1125x over previous
"""Trainium2 Bass kernel for nn_ClassLoss_11828339933550.

YOLO-style classification loss over 3 scales:
  loss = sum_s sum_b CE_mean(log_softmax(out_s[b,...,5:]), gt_scatter(targets[b])) / B

Strategy (data-parallel over batch, 2 batches per core on 8 cores):
  Host: build per-scale ground-truth class maps from `targets` (tiny [16,100,5]
  tensor, last-wins scatter), derive a per-row weight vector w (1/denom at
  masked rows, else 0) plus a compact list of (masked row, class) pairs.
  Device (per core, streaming all of its 41 MB shard):
    - stream pred rows [128, K*80] tiles; ACT exp in-place; DVE grouped
      reduce -> per-row sumexp
    - ACT ln over the per-row sumexp buffer; DVE tensor_tensor_reduce with the
      streamed w vector -> S1 = sum_r w_r * lse_r
    - compact gathered masked rows: one-hot select via iota==cls, weighted;
      TTR -> S2 = sum_r w_r * pred[r, cls_r]
  Host: loss = sum_cores(S1 - S2) / B.
"""

import ml_dtypes
import numpy as np

import concourse.bass as bass
import concourse.tile as tile
from concourse import mybir
from concourse.bass_utils import run_bass_kernel_spmd

# Problem constants (hardcoded per spec nn_ClassLoss_11828339933550)
B, T, A, C = 16, 100, 3, 80
GRIDS = (128, 64, 32)
IGNORE = -100
NCORES = 8
BPC = B // NCORES  # batches per core = 2

ROWS_PER_BATCH = A * sum(g * g for g in GRIDS)  # 64512
ROWS_PER_CORE = BPC * ROWS_PER_BATCH  # 129024
P = 128
K = 48  # rows per partition per tile
F = K * C  # 3840 floats per partition per tile
NT = ROWS_PER_CORE // (P * K)  # 21 tiles
NTW = NT * K  # 1008 per-row columns per partition
NG = 16  # compact gather tiles of [128, C] -> capacity 2048 masked rows

_DT = mybir.dt.float32
# Streamed logits travel as bf16: halves HBM traffic, and 16-bit dtypes let
# the DVE grouped reduce run in 2x mode. The lse accumulation stays fp32
# internally (DVE/ACT compute fp32); the tiny gathered class-logit path stays
# full fp32. Measured end-to-end rel err stays ~1e-5.
_DT_X = mybir.dt.bfloat16

LAST_RESULTS = None  # debugging: last BassKernelResults (used by test.py)

# The walrus build in this container encodes at most _MAXW sync-wait commands
# per instruction ("Too many sync wait commands" in codegen otherwise). The
# Tile scheduler merges waits onto single instructions (e.g. the kernel-tail
# drain waits on every DMA semaphore at once), so split any excess waits onto
# preceding wait-only NoOps on the same engine — the sequencer executes them
# in order, which is semantically identical.
_MAXW = 1


def _split_excess_waits(bir: bytes) -> bytes:
    import json as _json

    m = _json.loads(bir)
    n = 0
    for fn in m["functions"]:
        for bb in fn["blocks"]:
            new_instrs = []
            for ins in bb.get("instructions", []):
                si = ins.get("sync_info")
                waits = (si or {}).get("on_wait") or []
                if si is not None and len(waits) > _MAXW:
                    excess = waits[:-_MAXW]
                    si["on_wait"] = waits[-_MAXW:]
                    for i in range(0, len(excess), _MAXW):
                        n += 1
                        new_instrs.append(
                            {
                                "engine": ins["engine"],
                                "ins": [],
                                "outs": [],
                                "name": f"waitsplit-{n}",
                                "opcode": "NoOp",
                                "sync_info": {
                                    "on_update": [],
                                    "on_wait": excess[i : i + _MAXW],
                                },
                            }
                        )
                new_instrs.append(ins)
            bb["instructions"] = new_instrs
    return _json.dumps(m).encode()


class _Bass(bass.Bass):
    def to_json_bytes(self):
        return _split_excess_waits(super().to_json_bytes())


def _build_gt_flat(targets_b, H, W):
    """Per-batch gt map -> flattened (H, W, A) class vector, IGNORE elsewhere."""
    valid = ~np.all(targets_b == 0.0, axis=1)
    rows = (targets_b[:, 2] * H).astype(np.int32)
    cols = (targets_b[:, 1] * W).astype(np.int32)
    cls = targets_b[:, 0].astype(np.int32)
    gt = np.full((H, W), IGNORE, dtype=np.int32)
    idx = np.where(valid)[0]
    gt[rows[idx], cols[idx]] = cls[idx]  # sequential last-wins, like index_put_
    return np.broadcast_to(gt[:, :, None], (H, W, A)).reshape(-1)


def _build_kernel():
    nc = _Bass("TRN2", target_bir_lowering=False, debug=False)

    xs = nc.declare_dram_parameter("xs", [NT * P, F], _DT_X, isOutput=False)
    wt = nc.declare_dram_parameter("wt", [P, NTW], _DT, isOutput=False)
    gp = nc.declare_dram_parameter("gp", [NG * P, C], _DT, isOutput=False)
    gc = nc.declare_dram_parameter("gc", [P, NG], _DT, isOutput=False)
    gw = nc.declare_dram_parameter("gw", [P, NG], _DT, isOutput=False)
    res = nc.declare_dram_parameter("res", [P, 2], _DT, isOutput=True)

    with tile.TileContext(nc) as tc:
        with (
            tc.tile_pool(name="singles", bufs=1) as singles,
            tc.tile_pool(name="xpool", bufs=4) as xpool,
            tc.tile_pool(name="gpool", bufs=2) as gpool,
        ):
            sebuf = singles.tile([P, NTW], _DT_X)
            logse = singles.tile([P, NTW], _DT)
            wt_sb = singles.tile([P, NTW], _DT)
            gc_sb = singles.tile([P, NG], _DT)
            gw_sb = singles.tile([P, NG], _DT)
            s2buf = singles.tile([P, NG], _DT)
            iota80 = singles.tile([P, C], _DT)
            restile = singles.tile([P, 2], _DT)
            scrbig = singles.tile([P, NTW], _DT)

            nc.sync.dma_start(out=wt_sb[:], in_=wt[:, :])
            nc.sync.dma_start(out=gc_sb[:], in_=gc[:, :])
            nc.sync.dma_start(out=gw_sb[:], in_=gw[:, :])
            nc.gpsimd.iota(
                iota80[:],
                pattern=[[1, C]],
                base=0,
                channel_multiplier=0,
                allow_small_or_imprecise_dtypes=True,
            )

            # Main stream: per-row sumexp of exp(logits). TENSOR_REDUCE only
            # runs in 1x DVE mode, while bf16 TENSOR_TENSOR gets 2x — so
            # tree-halve the 80-wide groups twice with TT adds (2x) before a
            # 20-wide reduce (1x): ~2.9us vs ~4.1us of DVE time per tile.
            for t in range(NT):
                xtile = xpool.tile([P, F], _DT_X)
                nc.sync.dma_start(out=xtile[:], in_=xs[t * P : (t + 1) * P, :])
                nc.scalar.activation(
                    out=xtile[:], in_=xtile[:], func=mybir.ActivationFunctionType.Exp
                )
                x3 = xtile[:].rearrange("p (k c) -> p k c", k=K)
                h1 = xpool.tile([P, K, C // 2], _DT_X, tag="h1")
                nc.vector.tensor_tensor(
                    out=h1[:],
                    in0=x3[:, :, 0 : C // 2],
                    in1=x3[:, :, C // 2 : C],
                    op=mybir.AluOpType.add,
                )
                with nc.allow_low_precision(reason="bf16 sumexp store, fp32 accum"):
                    nc.vector.tensor_reduce(
                        out=sebuf[:, t * K : (t + 1) * K],
                        in_=h1[:],
                        axis=mybir.AxisListType.X,
                        op=mybir.AluOpType.add,
                    )

            # Compact select: S2 contributions per gathered masked row
            for j in range(NG):
                gtile = gpool.tile([P, C], _DT, tag="gtile")
                nc.sync.dma_start(out=gtile[:], in_=gp[j * P : (j + 1) * P, :])
                woh = gpool.tile([P, C], _DT, tag="woh")
                nc.vector.tensor_scalar(
                    out=woh[:],
                    in0=iota80[:],
                    scalar1=gc_sb[:, j : j + 1],
                    scalar2=gw_sb[:, j : j + 1],
                    op0=mybir.AluOpType.is_equal,
                    op1=mybir.AluOpType.mult,
                )
                scr = gpool.tile([P, C], _DT, tag="scr")
                nc.vector.tensor_tensor(
                    out=scr[:],
                    in0=woh[:],
                    in1=gtile[:],
                    op=mybir.AluOpType.mult,
                )
                nc.vector.tensor_reduce(
                    out=s2buf[:, j : j + 1],
                    in_=scr[:],
                    axis=mybir.AxisListType.X,
                    op=mybir.AluOpType.add,
                )

            # Final: S1 = sum w * ln(sumexp); S2 = sum of compact accums
            nc.scalar.activation(
                out=logse[:], in_=sebuf[:], func=mybir.ActivationFunctionType.Ln
            )
            nc.vector.tensor_tensor(
                out=scrbig[:],
                in0=logse[:],
                in1=wt_sb[:],
                op=mybir.AluOpType.mult,
            )
            nc.vector.tensor_reduce(
                out=restile[:, 0:1],
                in_=scrbig[:],
                axis=mybir.AxisListType.X,
                op=mybir.AluOpType.add,
            )
            nc.vector.tensor_reduce(
                out=restile[:, 1:2],
                in_=s2buf[:],
                axis=mybir.AxisListType.X,
                op=mybir.AluOpType.add,
            )
            nc.sync.dma_start(out=res[:, :], in_=restile[:])

    return nc


def _prep_core_inputs(core, outs, targets):
    """Build the per-core input map (pred shard + weights + compact gather)."""
    pred_segs = []
    w_segs = []
    cls_segs = []
    for b in range(BPC * core, BPC * core + BPC):
        for si, H in enumerate(GRIDS):
            o = outs[si][b]  # [A, H, W, 85]
            pred_segs.append(np.ascontiguousarray(o[..., 5:]).reshape(-1, C))
            gt_flat = _build_gt_flat(targets[b], H, H)
            mask = gt_flat != IGNORE
            denom = max(int(mask.sum()), 1)
            w_segs.append(mask.astype(np.float32) / np.float32(denom))
            cls_segs.append(gt_flat)

    pred = np.concatenate(pred_segs, axis=0)  # [ROWS_PER_CORE, C] f32
    w_flat = np.concatenate(w_segs)  # [ROWS_PER_CORE]
    cls_flat = np.concatenate(cls_segs)  # [ROWS_PER_CORE] int32 (IGNORE at unmasked)

    xs = np.ascontiguousarray(pred.reshape(NT * P, F)).astype(ml_dtypes.bfloat16)
    wt = np.ascontiguousarray(
        w_flat.reshape(NT, P, K).transpose(1, 0, 2).reshape(P, NTW)
    )

    midx = np.where(w_flat > 0)[0]
    nm = len(midx)
    assert nm <= NG * P, f"masked rows {nm} exceed compact capacity"
    gp = np.zeros((NG * P, C), dtype=np.float32)
    gp[:nm] = pred[midx]
    gcw = np.zeros(NG * P, dtype=np.float32)
    gcl = np.zeros(NG * P, dtype=np.float32)
    gcw[:nm] = w_flat[midx]
    gcl[:nm] = cls_flat[midx].astype(np.float32)
    gc = np.ascontiguousarray(gcl.reshape(NG, P).T)
    gw = np.ascontiguousarray(gcw.reshape(NG, P).T)

    return {"xs": xs, "wt": wt, "gp": gp, "gc": gc, "gw": gw}


def kernel(out0, out1, out2, targets):
    out0 = np.asarray(out0, dtype=np.float32)
    out1 = np.asarray(out1, dtype=np.float32)
    out2 = np.asarray(out2, dtype=np.float32)
    targets = np.asarray(targets, dtype=np.float32)
    outs = (out0, out1, out2)

    in_maps = [_prep_core_inputs(c, outs, targets) for c in range(NCORES)]

    nc = _build_kernel()
    br = run_bass_kernel_spmd(nc, in_maps, list(range(NCORES)))
    global LAST_RESULTS
    LAST_RESULTS = br
    results = br.results

    total = 0.0
    for c in range(NCORES):
        r = np.asarray(results[c]["res"], dtype=np.float64)
        total += r[:, 0].sum() - r[:, 1].sum()
    return np.asarray(total / B, dtype=np.float32)


# revision 19
# speedup vs baseline: 1.2599x; 1.0564x over previous
"""Trainium2 Bass kernel for nn_ClassLoss_11828339933550.

YOLO-style classification loss over 3 scales:
  loss = sum_s sum_b CE_mean(log_softmax(out_s[b,...,5:]), gt_scatter(targets[b])) / B

Strategy (data-parallel over batch, 2 batches per core on 8 cores):
  Host: build per-scale ground-truth class maps from `targets` (tiny [16,100,5]
  tensor, last-wins scatter), derive a per-row weight vector w (1/denom at
  masked rows, else 0) plus a compact list of (masked row, class) pairs.
  Device (per core, streaming all of its 41 MB shard):
    - stream pred rows [128, K*80] tiles; ACT exp in-place; DVE grouped
      reduce -> per-row sumexp
    - ACT ln over the per-row sumexp buffer; DVE tensor_tensor_reduce with the
      streamed w vector -> S1 = sum_r w_r * lse_r
    - compact gathered masked rows: one-hot select via iota==cls, weighted;
      TTR -> S2 = sum_r w_r * pred[r, cls_r]
  Host: loss = sum_cores(S1 - S2) / B.
"""

import ml_dtypes
import numpy as np

import concourse.bass as bass
import concourse.tile as tile
from concourse import mybir
from concourse.bass_utils import run_bass_kernel_spmd

# Problem constants (hardcoded per spec nn_ClassLoss_11828339933550)
B, T, A, C = 16, 100, 3, 80
GRIDS = (128, 64, 32)
IGNORE = -100
NCORES = 8
BPC = B // NCORES  # batches per core = 2

ROWS_PER_BATCH = A * sum(g * g for g in GRIDS)  # 64512
ROWS_PER_CORE = BPC * ROWS_PER_BATCH  # 129024
P = 128
K = 48  # rows per partition per tile
F = K * C  # 3840 floats per partition per tile
NT = ROWS_PER_CORE // (P * K)  # 21 tiles
NTW = NT * K  # 1008 per-row columns per partition
NG = 16  # compact gather tiles of [128, C] -> capacity 2048 masked rows

_DT = mybir.dt.float32
# Streamed logits travel as bf16: halves HBM traffic, and 16-bit dtypes let
# the DVE grouped reduce run in 2x mode. The lse accumulation stays fp32
# internally (DVE/ACT compute fp32); the tiny gathered class-logit path stays
# full fp32. Measured end-to-end rel err stays ~1e-5.
_DT_X = mybir.dt.bfloat16

LAST_RESULTS = None  # debugging: last BassKernelResults (used by test.py)

# The walrus build in this container encodes at most _MAXW sync-wait commands
# per instruction ("Too many sync wait commands" in codegen otherwise). The
# Tile scheduler merges waits onto single instructions (e.g. the kernel-tail
# drain waits on every DMA semaphore at once), so split any excess waits onto
# preceding wait-only NoOps on the same engine — the sequencer executes them
# in order, which is semantically identical.
_MAXW = 1


def _split_excess_waits(bir: bytes) -> bytes:
    import json as _json

    m = _json.loads(bir)
    n = 0
    for fn in m["functions"]:
        for bb in fn["blocks"]:
            new_instrs = []
            for ins in bb.get("instructions", []):
                si = ins.get("sync_info")
                waits = (si or {}).get("on_wait") or []
                if si is not None and len(waits) > _MAXW:
                    excess = waits[:-_MAXW]
                    si["on_wait"] = waits[-_MAXW:]
                    for i in range(0, len(excess), _MAXW):
                        n += 1
                        new_instrs.append(
                            {
                                "engine": ins["engine"],
                                "ins": [],
                                "outs": [],
                                "name": f"waitsplit-{n}",
                                "opcode": "NoOp",
                                "sync_info": {
                                    "on_update": [],
                                    "on_wait": excess[i : i + _MAXW],
                                },
                            }
                        )
                new_instrs.append(ins)
            bb["instructions"] = new_instrs
    return _json.dumps(m).encode()


class _Bass(bass.Bass):
    def to_json_bytes(self):
        return _split_excess_waits(super().to_json_bytes())


def _build_gt_flat(targets_b, H, W):
    """Per-batch gt map -> flattened (H, W, A) class vector, IGNORE elsewhere."""
    valid = ~np.all(targets_b == 0.0, axis=1)
    rows = (targets_b[:, 2] * H).astype(np.int32)
    cols = (targets_b[:, 1] * W).astype(np.int32)
    cls = targets_b[:, 0].astype(np.int32)
    gt = np.full((H, W), IGNORE, dtype=np.int32)
    idx = np.where(valid)[0]
    gt[rows[idx], cols[idx]] = cls[idx]  # sequential last-wins, like index_put_
    return np.broadcast_to(gt[:, :, None], (H, W, A)).reshape(-1)


def _build_kernel():
    nc = _Bass("TRN2", target_bir_lowering=False, debug=False)

    xs = nc.declare_dram_parameter("xs", [NT * P, F], _DT_X, isOutput=False)
    wt = nc.declare_dram_parameter("wt", [P, NTW], _DT, isOutput=False)
    gp = nc.declare_dram_parameter("gp", [NG * P, C], _DT, isOutput=False)
    gc = nc.declare_dram_parameter("gc", [P, NG], _DT, isOutput=False)
    gw = nc.declare_dram_parameter("gw", [P, NG], _DT, isOutput=False)
    res = nc.declare_dram_parameter("res", [P, 2], _DT, isOutput=True)

    with tile.TileContext(nc) as tc:
        with (
            tc.tile_pool(name="singles", bufs=1) as singles,
            tc.tile_pool(name="xpool", bufs=4) as xpool,
            tc.tile_pool(name="gpool", bufs=2) as gpool,
        ):
            sebuf = singles.tile([P, NTW], _DT_X)
            logse = singles.tile([P, NTW], _DT)
            wt_sb = singles.tile([P, NTW], _DT)
            gc_sb = singles.tile([P, NG], _DT)
            gw_sb = singles.tile([P, NG], _DT)
            s2buf = singles.tile([P, NG], _DT)
            iota80 = singles.tile([P, C], _DT)
            restile = singles.tile([P, 2], _DT)
            scrbig = singles.tile([P, NTW], _DT)

            nc.sync.dma_start(out=wt_sb[:], in_=wt[:, :])
            nc.sync.dma_start(out=gc_sb[:], in_=gc[:, :])
            nc.sync.dma_start(out=gw_sb[:], in_=gw[:, :])
            nc.gpsimd.iota(
                iota80[:],
                pattern=[[1, C]],
                base=0,
                channel_multiplier=0,
                allow_small_or_imprecise_dtypes=True,
            )

            # Main stream: per-row sumexp of exp(logits). TENSOR_REDUCE only
            # runs in 1x DVE mode, and TT only reaches 2x with fully
            # contiguous step-1 APs — so the host ships each tile's columns
            # as four class-quarter blocks [Q0|Q1|Q2|Q3]; two flat contiguous
            # bf16 TT adds (2x) fold 80 -> 20 wide, then a 20-wide grouped
            # reduce (1x) finishes: ~2.9us vs ~4.1us of DVE time per tile.
            for t in range(NT):
                xtile = xpool.tile([P, F], _DT_X)
                nc.sync.dma_start(out=xtile[:], in_=xs[t * P : (t + 1) * P, :])
                nc.scalar.activation(
                    out=xtile[:], in_=xtile[:], func=mybir.ActivationFunctionType.Exp
                )
                h1 = xpool.tile([P, F // 2], _DT_X, tag="h1")
                nc.vector.tensor_tensor(
                    out=h1[:],
                    in0=xtile[:, 0 : F // 2],
                    in1=xtile[:, F // 2 : F],
                    op=mybir.AluOpType.add,
                )
                h2 = xpool.tile([P, F // 4], _DT_X, tag="h2")
                nc.vector.tensor_tensor(
                    out=h2[:],
                    in0=h1[:, 0 : F // 4],
                    in1=h1[:, F // 4 : F // 2],
                    op=mybir.AluOpType.add,
                )
                with nc.allow_low_precision(reason="bf16 sumexp store, fp32 accum"):
                    nc.vector.tensor_reduce(
                        out=sebuf[:, t * K : (t + 1) * K],
                        in_=h2[:].rearrange("p (k c) -> p k c", k=K),
                        axis=mybir.AxisListType.X,
                        op=mybir.AluOpType.add,
                    )

            # Compact select: S2 contributions per gathered masked row
            for j in range(NG):
                gtile = gpool.tile([P, C], _DT, tag="gtile")
                nc.sync.dma_start(out=gtile[:], in_=gp[j * P : (j + 1) * P, :])
                woh = gpool.tile([P, C], _DT, tag="woh")
                nc.vector.tensor_scalar(
                    out=woh[:],
                    in0=iota80[:],
                    scalar1=gc_sb[:, j : j + 1],
                    scalar2=gw_sb[:, j : j + 1],
                    op0=mybir.AluOpType.is_equal,
                    op1=mybir.AluOpType.mult,
                )
                scr = gpool.tile([P, C], _DT, tag="scr")
                nc.vector.tensor_tensor(
                    out=scr[:],
                    in0=woh[:],
                    in1=gtile[:],
                    op=mybir.AluOpType.mult,
                )
                nc.vector.tensor_reduce(
                    out=s2buf[:, j : j + 1],
                    in_=scr[:],
                    axis=mybir.AxisListType.X,
                    op=mybir.AluOpType.add,
                )

            # Final: S1 = sum w * ln(sumexp); S2 = sum of compact accums
            nc.scalar.activation(
                out=logse[:], in_=sebuf[:], func=mybir.ActivationFunctionType.Ln
            )
            nc.vector.tensor_tensor(
                out=scrbig[:],
                in0=logse[:],
                in1=wt_sb[:],
                op=mybir.AluOpType.mult,
            )
            nc.vector.tensor_reduce(
                out=restile[:, 0:1],
                in_=scrbig[:],
                axis=mybir.AxisListType.X,
                op=mybir.AluOpType.add,
            )
            nc.vector.tensor_reduce(
                out=restile[:, 1:2],
                in_=s2buf[:],
                axis=mybir.AxisListType.X,
                op=mybir.AluOpType.add,
            )
            nc.sync.dma_start(out=res[:, :], in_=restile[:])

    return nc


def _prep_core_inputs(core, outs, targets):
    """Build the per-core input map (pred shard + weights + compact gather)."""
    pred_segs = []
    w_segs = []
    cls_segs = []
    for b in range(BPC * core, BPC * core + BPC):
        for si, H in enumerate(GRIDS):
            o = outs[si][b]  # [A, H, W, 85]
            pred_segs.append(np.ascontiguousarray(o[..., 5:]).reshape(-1, C))
            gt_flat = _build_gt_flat(targets[b], H, H)
            mask = gt_flat != IGNORE
            denom = max(int(mask.sum()), 1)
            w_segs.append(mask.astype(np.float32) / np.float32(denom))
            cls_segs.append(gt_flat)

    pred = np.concatenate(pred_segs, axis=0)  # [ROWS_PER_CORE, C] f32
    w_flat = np.concatenate(w_segs)  # [ROWS_PER_CORE]
    cls_flat = np.concatenate(cls_segs)  # [ROWS_PER_CORE] int32 (IGNORE at unmasked)

    # Tile columns as four class-quarter blocks so the kernel's TT halving
    # adds read fully contiguous APs: col = q*(K*C//4) + k*(C//4) + c.
    xs = np.ascontiguousarray(
        pred.reshape(NT, P, K, 4, C // 4)
        .transpose(0, 1, 3, 2, 4)
        .reshape(NT * P, F)
    ).astype(ml_dtypes.bfloat16)
    wt = np.ascontiguousarray(
        w_flat.reshape(NT, P, K).transpose(1, 0, 2).reshape(P, NTW)
    )

    midx = np.where(w_flat > 0)[0]
    nm = len(midx)
    assert nm <= NG * P, f"masked rows {nm} exceed compact capacity"
    gp = np.zeros((NG * P, C), dtype=np.float32)
    gp[:nm] = pred[midx]
    gcw = np.zeros(NG * P, dtype=np.float32)
    gcl = np.zeros(NG * P, dtype=np.float32)
    gcw[:nm] = w_flat[midx]
    gcl[:nm] = cls_flat[midx].astype(np.float32)
    gc = np.ascontiguousarray(gcl.reshape(NG, P).T)
    gw = np.ascontiguousarray(gcw.reshape(NG, P).T)

    return {"xs": xs, "wt": wt, "gp": gp, "gc": gc, "gw": gw}


def kernel(out0, out1, out2, targets):
    out0 = np.asarray(out0, dtype=np.float32)
    out1 = np.asarray(out1, dtype=np.float32)
    out2 = np.asarray(out2, dtype=np.float32)
    targets = np.asarray(targets, dtype=np.float32)
    outs = (out0, out1, out2)

    in_maps = [_prep_core_inputs(c, outs, targets) for c in range(NCORES)]

    nc = _build_kernel()
    br = run_bass_kernel_spmd(nc, in_maps, list(range(NCORES)))
    global LAST_RESULTS
    LAST_RESULTS = br
    results = br.results

    total = 0.0
    for c in range(NCORES):
        r = np.asarray(results[c]["res"], dtype=np.float64)
        total += r[:, 0].sum() - r[:, 1].sum()
    return np.asarray(total / B, dtype=np.float32)


# revision 22
# speedup vs baseline: 1.4119x; 1.1206x over previous
"""Trainium2 Bass kernel for nn_ClassLoss_11828339933550.

YOLO-style classification loss over 3 scales:
  loss = sum_s sum_b CE_mean(log_softmax(out_s[b,...,5:]), gt_scatter(targets[b])) / B

Strategy (data-parallel over batch, 2 batches per core on 8 cores):
  Host: build per-scale ground-truth class maps from `targets` (tiny [16,100,5]
  tensor, last-wins scatter), derive a per-row weight vector w (1/denom at
  masked rows, else 0) plus a compact list of (masked row, class) pairs.
  Device (per core, streaming all of its 41 MB shard):
    - stream pred rows [128, K*80] tiles; ACT exp in-place; DVE grouped
      reduce -> per-row sumexp
    - ACT ln over the per-row sumexp buffer; DVE tensor_tensor_reduce with the
      streamed w vector -> S1 = sum_r w_r * lse_r
    - compact gathered masked rows: one-hot select via iota==cls, weighted;
      TTR -> S2 = sum_r w_r * pred[r, cls_r]
  Host: loss = sum_cores(S1 - S2) / B.
"""

import ml_dtypes
import numpy as np

import concourse.bass as bass
import concourse.tile as tile
from concourse import mybir
from concourse.bass_utils import run_bass_kernel_spmd

# Problem constants (hardcoded per spec nn_ClassLoss_11828339933550)
B, T, A, C = 16, 100, 3, 80
GRIDS = (128, 64, 32)
IGNORE = -100
NCORES = 8
BPC = B // NCORES  # batches per core = 2

ROWS_PER_BATCH = A * sum(g * g for g in GRIDS)  # 64512
ROWS_PER_CORE = BPC * ROWS_PER_BATCH  # 129024
P = 128
K = 48  # rows per partition per tile
F = K * C  # 3840 floats per partition per tile
NT = ROWS_PER_CORE // (P * K)  # 21 tiles
NTW = NT * K  # 1008 per-row columns per partition
NG = 16  # compact gather tiles of [128, C] -> capacity 2048 masked rows

_DT = mybir.dt.float32
# Streamed logits travel as bf16: halves HBM traffic, and 16-bit dtypes let
# the DVE grouped reduce run in 2x mode. The lse accumulation stays fp32
# internally (DVE/ACT compute fp32); the tiny gathered class-logit path stays
# full fp32. Measured end-to-end rel err stays ~1e-5.
_DT_X = mybir.dt.bfloat16

LAST_RESULTS = None  # debugging: last BassKernelResults (used by test.py)

# The walrus build in this container encodes at most _MAXW sync-wait commands
# per instruction ("Too many sync wait commands" in codegen otherwise). The
# Tile scheduler merges waits onto single instructions (e.g. the kernel-tail
# drain waits on every DMA semaphore at once), so split any excess waits onto
# preceding wait-only NoOps on the same engine — the sequencer executes them
# in order, which is semantically identical.
_MAXW = 1


def _split_excess_waits(bir: bytes) -> bytes:
    import json as _json

    m = _json.loads(bir)
    n = 0
    for fn in m["functions"]:
        for bb in fn["blocks"]:
            new_instrs = []
            for ins in bb.get("instructions", []):
                si = ins.get("sync_info")
                waits = (si or {}).get("on_wait") or []
                if si is not None and len(waits) > _MAXW:
                    excess = waits[:-_MAXW]
                    si["on_wait"] = waits[-_MAXW:]
                    for i in range(0, len(excess), _MAXW):
                        n += 1
                        new_instrs.append(
                            {
                                "engine": ins["engine"],
                                "ins": [],
                                "outs": [],
                                "name": f"waitsplit-{n}",
                                "opcode": "NoOp",
                                "sync_info": {
                                    "on_update": [],
                                    "on_wait": excess[i : i + _MAXW],
                                },
                            }
                        )
                new_instrs.append(ins)
            bb["instructions"] = new_instrs
    return _json.dumps(m).encode()


class _Bass(bass.Bass):
    def to_json_bytes(self):
        return _split_excess_waits(super().to_json_bytes())


def _build_gt_flat(targets_b, H, W):
    """Per-batch gt map -> flattened (H, W, A) class vector, IGNORE elsewhere."""
    valid = ~np.all(targets_b == 0.0, axis=1)
    rows = (targets_b[:, 2] * H).astype(np.int32)
    cols = (targets_b[:, 1] * W).astype(np.int32)
    cls = targets_b[:, 0].astype(np.int32)
    gt = np.full((H, W), IGNORE, dtype=np.int32)
    idx = np.where(valid)[0]
    gt[rows[idx], cols[idx]] = cls[idx]  # sequential last-wins, like index_put_
    return np.broadcast_to(gt[:, :, None], (H, W, A)).reshape(-1)


def _build_kernel():
    nc = _Bass("TRN2", target_bir_lowering=False, debug=False)

    xs = nc.declare_dram_parameter("xs", [NT * P, F], _DT_X, isOutput=False)
    wt = nc.declare_dram_parameter("wt", [P, NTW], _DT, isOutput=False)
    gp = nc.declare_dram_parameter("gp", [NG * P, C], _DT, isOutput=False)
    gc = nc.declare_dram_parameter("gc", [P, NG], _DT, isOutput=False)
    gw = nc.declare_dram_parameter("gw", [P, NG], _DT, isOutput=False)
    res = nc.declare_dram_parameter("res", [P, 2], _DT, isOutput=True)

    with tile.TileContext(nc) as tc:
        with (
            tc.tile_pool(name="singles", bufs=1) as singles,
            tc.tile_pool(name="xpool", bufs=6) as xpool,
            tc.tile_pool(name="gpool", bufs=4) as gpool,
        ):
            sebuf = singles.tile([P, NTW], _DT_X)
            logse = singles.tile([P, NTW], _DT)
            wt_sb = singles.tile([P, NTW], _DT)
            gc_sb = singles.tile([P, NG], _DT)
            gw_sb = singles.tile([P, NG], _DT)
            s2buf = singles.tile([P, NG], _DT)
            iota80 = singles.tile([P, C], _DT)
            restile = singles.tile([P, 2], _DT)
            scrbig = singles.tile([P, NTW], _DT)

            # Main stream: per-row sumexp of exp(logits). TENSOR_REDUCE only
            # runs in 1x DVE mode, and TT only reaches 2x with fully
            # contiguous step-1 APs — so the host ships each tile's columns
            # as four class-quarter blocks [Q0|Q1|Q2|Q3]; two flat contiguous
            # bf16 TT adds (2x) fold 80 -> 20 wide, then a 20-wide grouped
            # reduce (1x) finishes: ~2.9us vs ~4.1us of DVE time per tile.
            for t in range(NT):
                xtile = xpool.tile([P, F], _DT_X)
                nc.sync.dma_start(out=xtile[:], in_=xs[t * P : (t + 1) * P, :])
                nc.scalar.activation(
                    out=xtile[:], in_=xtile[:], func=mybir.ActivationFunctionType.Exp
                )
                h1 = xpool.tile([P, F // 2], _DT_X, tag="h1")
                nc.vector.tensor_tensor(
                    out=h1[:],
                    in0=xtile[:, 0 : F // 2],
                    in1=xtile[:, F // 2 : F],
                    op=mybir.AluOpType.add,
                )
                h2 = xpool.tile([P, F // 4], _DT_X, tag="h2")
                nc.vector.tensor_tensor(
                    out=h2[:],
                    in0=h1[:, 0 : F // 4],
                    in1=h1[:, F // 4 : F // 2],
                    op=mybir.AluOpType.add,
                )
                with nc.allow_low_precision(reason="bf16 sumexp store, fp32 accum"):
                    nc.vector.tensor_reduce(
                        out=sebuf[:, t * K : (t + 1) * K],
                        in_=h2[:].rearrange("p (k c) -> p k c", k=K),
                        axis=mybir.AxisListType.X,
                        op=mybir.AluOpType.add,
                    )

            # Aux loads issued after the stream so the first pred tiles win
            # the DMA queues and ACT starts as early as possible.
            nc.sync.dma_start(out=gc_sb[:], in_=gc[:, :])
            nc.sync.dma_start(out=gw_sb[:], in_=gw[:, :])
            nc.sync.dma_start(out=wt_sb[:], in_=wt[:, :])
            nc.gpsimd.iota(
                iota80[:],
                pattern=[[1, C]],
                base=0,
                channel_multiplier=0,
                allow_small_or_imprecise_dtypes=True,
            )

            # Compact select: S2 contributions per gathered masked row
            for j in range(NG):
                gtile = gpool.tile([P, C], _DT, tag="gtile")
                nc.sync.dma_start(out=gtile[:], in_=gp[j * P : (j + 1) * P, :])
                woh = gpool.tile([P, C], _DT, tag="woh")
                nc.vector.tensor_scalar(
                    out=woh[:],
                    in0=iota80[:],
                    scalar1=gc_sb[:, j : j + 1],
                    scalar2=gw_sb[:, j : j + 1],
                    op0=mybir.AluOpType.is_equal,
                    op1=mybir.AluOpType.mult,
                )
                scr = gpool.tile([P, C], _DT, tag="scr")
                nc.vector.tensor_tensor(
                    out=scr[:],
                    in0=woh[:],
                    in1=gtile[:],
                    op=mybir.AluOpType.mult,
                )
                nc.vector.tensor_reduce(
                    out=s2buf[:, j : j + 1],
                    in_=scr[:],
                    axis=mybir.AxisListType.X,
                    op=mybir.AluOpType.add,
                )

            # Final: S1 = sum w * ln(sumexp); S2 = sum of compact accums
            nc.scalar.activation(
                out=logse[:], in_=sebuf[:], func=mybir.ActivationFunctionType.Ln
            )
            nc.vector.tensor_tensor(
                out=scrbig[:],
                in0=logse[:],
                in1=wt_sb[:],
                op=mybir.AluOpType.mult,
            )
            nc.vector.tensor_reduce(
                out=restile[:, 0:1],
                in_=scrbig[:],
                axis=mybir.AxisListType.X,
                op=mybir.AluOpType.add,
            )
            nc.vector.tensor_reduce(
                out=restile[:, 1:2],
                in_=s2buf[:],
                axis=mybir.AxisListType.X,
                op=mybir.AluOpType.add,
            )
            nc.sync.dma_start(out=res[:, :], in_=restile[:])

    return nc


def _prep_core_inputs(core, outs, targets):
    """Build the per-core input map (pred shard + weights + compact gather)."""
    pred_segs = []
    w_segs = []
    cls_segs = []
    for b in range(BPC * core, BPC * core + BPC):
        for si, H in enumerate(GRIDS):
            o = outs[si][b]  # [A, H, W, 85]
            pred_segs.append(np.ascontiguousarray(o[..., 5:]).reshape(-1, C))
            gt_flat = _build_gt_flat(targets[b], H, H)
            mask = gt_flat != IGNORE
            denom = max(int(mask.sum()), 1)
            w_segs.append(mask.astype(np.float32) / np.float32(denom))
            cls_segs.append(gt_flat)

    pred = np.concatenate(pred_segs, axis=0)  # [ROWS_PER_CORE, C] f32
    w_flat = np.concatenate(w_segs)  # [ROWS_PER_CORE]
    cls_flat = np.concatenate(cls_segs)  # [ROWS_PER_CORE] int32 (IGNORE at unmasked)

    # Tile columns as four class-quarter blocks so the kernel's TT halving
    # adds read fully contiguous APs: col = q*(K*C//4) + k*(C//4) + c.
    xs = np.ascontiguousarray(
        pred.reshape(NT, P, K, 4, C // 4)
        .transpose(0, 1, 3, 2, 4)
        .reshape(NT * P, F)
    ).astype(ml_dtypes.bfloat16)
    wt = np.ascontiguousarray(
        w_flat.reshape(NT, P, K).transpose(1, 0, 2).reshape(P, NTW)
    )

    midx = np.where(w_flat > 0)[0]
    nm = len(midx)
    assert nm <= NG * P, f"masked rows {nm} exceed compact capacity"
    gp = np.zeros((NG * P, C), dtype=np.float32)
    gp[:nm] = pred[midx]
    gcw = np.zeros(NG * P, dtype=np.float32)
    gcl = np.zeros(NG * P, dtype=np.float32)
    gcw[:nm] = w_flat[midx]
    gcl[:nm] = cls_flat[midx].astype(np.float32)
    gc = np.ascontiguousarray(gcl.reshape(NG, P).T)
    gw = np.ascontiguousarray(gcw.reshape(NG, P).T)

    return {"xs": xs, "wt": wt, "gp": gp, "gc": gc, "gw": gw}


def kernel(out0, out1, out2, targets):
    out0 = np.asarray(out0, dtype=np.float32)
    out1 = np.asarray(out1, dtype=np.float32)
    out2 = np.asarray(out2, dtype=np.float32)
    targets = np.asarray(targets, dtype=np.float32)
    outs = (out0, out1, out2)

    in_maps = [_prep_core_inputs(c, outs, targets) for c in range(NCORES)]

    nc = _build_kernel()
    br = run_bass_kernel_spmd(nc, in_maps, list(range(NCORES)))
    global LAST_RESULTS
    LAST_RESULTS = br
    results = br.results

    total = 0.0
    for c in range(NCORES):
        r = np.asarray(results[c]["res"], dtype=np.float64)
        total += r[:, 0].sum() - r[:, 1].sum()
    return np.asarray(total / B, dtype=np.float32)
